# revision 29
# baseline (speedup 1.0000x reference)
"""Trainium2 Bass kernel for GQA attention block (B=2, S=2048, HID=4096, 32Q/8KV heads).

Sharding: hybrid TP4 x DP2 over 8 NeuronCores.
  core c: batch b = c // 4, TP slice t = c % 4.
  Each core handles one batch element, 8 Q heads (2 KV heads); o_proj partials
  summed on host.

On-chip dataflow is feature-major: qT/kT [head_dim, tokens]; scores come out
keys-major (softmax key-sum via ones-matmul), attention output lands as o_proj
lhsT. RoPE = two SBUF partition-shift DMAs + vector combine (sign folded into
host-negated sin table).

Software pipeline (v2): attention's softmax exp saturates ScalarE (~285us of
EXP for the whole kernel), so Q-projection is interleaved with attention at
score-group granularity: token chunk c's Q-proj matmuls fill the PE while
chunk c-1's attention exp runs on ScalarE. The final chunk's attention
interleaves with o_proj tiles. All Q-evac/rope work rides on VectorE so
ScalarE does exp only. PSUM: pss(4) + po(1) + psb(1) + psq|pot(2) = 8 banks.
"""
import os
import sys

for _p in ("/opt/trn_rl_repo", "/root/.axon_site"):
    if _p not in sys.path and os.path.isdir(_p):
        sys.path.append(_p)

import numpy as np

B, S_FULL, HID = 2, 2048, 4096
NH, NKV, HD = 32, 8, 128
TP = 4                 # tensor-parallel ways
QH = NH // TP          # 8 q heads per core
KVH = NKV // TP        # 2 kv heads per core
FQ = QH * HD           # 1024
FKV = KVH * HD         # 256
KH = HID // 128        # 32 contraction tiles
SCALE = 1.0 / float(np.sqrt(HD))

last_exec_time_ns = None


def build_nc(S: int = S_FULL, dt: str = "bf16"):
    """Build the per-core Bass program (SPMD: same program, per-core inputs)."""
    import concourse.bass as bass
    import concourse.tile as tile
    from concourse import bacc, mybir
    from contextlib import ExitStack

    f32 = mybir.dt.float32
    mdt = mybir.dt.bfloat16 if dt == "bf16" else mybir.dt.float32r
    ddt = mdt if dt == "bf16" else f32   # DRAM dtype for big inputs
    TBP = min(1024, S)                   # kv projection token block
    NTB = S // TBP
    NB5 = [slice(j * 512, min((j + 1) * 512, TBP)) for j in range((TBP + 511) // 512)]
    KT = S // 128                        # key/token tiles
    HALF = HD // 2
    QB = min(512, S)                     # pipeline token chunk
    NCH = S // QB                        # 4 chunks
    NG = KT // 2                         # score groups per (head, chunk): 8

    nc = bacc.Bacc("TRN2", target_bir_lowering=False, debug=False)

    hsT = nc.dram_tensor("hsT", [HID, S], ddt, kind="ExternalInput")
    cosT = nc.dram_tensor("cosT", [HD, S], ddt, kind="ExternalInput")
    sinT = nc.dram_tensor("sinT", [HD, S], ddt, kind="ExternalInput")  # sign-folded
    wqh = nc.dram_tensor("wqh", [128, QH * HID], ddt, kind="ExternalInput")
    bq = nc.dram_tensor("bq", [QH, HD], f32, kind="ExternalInput")
    wkv = nc.dram_tensor("wkv", [HID, 2 * FKV], ddt, kind="ExternalInput")
    bk = nc.dram_tensor("bk", [KVH, HD], f32, kind="ExternalInput")
    bv = nc.dram_tensor("bv", [KVH, HD], f32, kind="ExternalInput")
    woh = nc.dram_tensor("woh", [128, (HID // 512) * QH * 512], ddt,
                         kind="ExternalInput")
    ident = nc.dram_tensor("ident", [128, 128], ddt, kind="ExternalInput")
    ones = nc.dram_tensor("ones", [128, 128], ddt, kind="ExternalInput")
    out = nc.dram_tensor("out", [S, HID], f32, kind="ExternalOutput")

    def bc(ap):
        """View a DRAM fp32 AP as f32r (no-op for bf16)."""
        return ap.bitcast(mdt) if dt == "f32r" else ap

    with tile.TileContext(nc) as tc, ExitStack() as ctx:
        Exp = mybir.ActivationFunctionType.Exp
        Ident = mybir.ActivationFunctionType.Identity

        const = ctx.enter_context(tc.tile_pool(name="const", bufs=1))
        bq_t = const.tile([128, QH], f32)
        bk_t = const.tile([128, KVH], f32)
        bv_t = const.tile([128, KVH], f32)
        id_t = const.tile([128, 128], mdt)
        ones_t = const.tile([128, 128], mdt)
        cos_t = const.tile([128, S], mdt)
        sin_t = const.tile([128, S], mdt)

        # Persistent activations (feature-major). attnT overwrites q in place.
        qpool = ctx.enter_context(tc.tile_pool(name="qpool", bufs=1))
        q_t = [qpool.tile([128, S], mdt, name=f"q{h}") for h in range(QH)]
        kvpool = ctx.enter_context(tc.tile_pool(name="kvpool", bufs=1))
        k_t = [kvpool.tile([128, S], mdt, name=f"k{f}") for f in range(KVH)]
        v_t = kvpool.tile([128, KT * FKV], mdt, name="v")  # [tok%128, (kt, kv*128+d)]

        rope_pool = ctx.enter_context(tc.tile_pool(name="ropep", bufs=1))

        def rope_inplace(x_t, sl, tag):
            """x[:, sl] = x[:, sl]*cos + shift64(x[:, sl])*sin', VectorE + DMA only."""
            w = sl.stop - sl.start
            rot = rope_pool.tile([128, 1024], mdt, name=f"rot_{tag}", tag="rot", bufs=4)
            nc.sync.dma_start(rot[0:HALF, :w], x_t[HALF:128, sl])
            nc.sync.dma_start(rot[HALF:128, :w], x_t[0:HALF, sl])
            t1 = rope_pool.tile([128, 1024], mdt, name=f"t1_{tag}", tag="t1", bufs=4)
            nc.vector.tensor_mul(t1[:, :w], rot[:, :w], sin_t[:, sl])
            nc.vector.tensor_mul(x_t[:, sl], x_t[:, sl], cos_t[:, sl])
            nc.vector.tensor_add(x_t[:, sl], x_t[:, sl], t1[:, :w])

        # Pipeline input pools opened early so chunk-0 prefetch can overlap
        # the V-transpose section.
        hsq = ctx.enter_context(tc.tile_pool(name="hsq", bufs=1))
        wqp = ctx.enter_context(tc.tile_pool(name="wqp", bufs=1))
        QB = min(512, S)
        NCH = S // QB
        NG = KT // 2

        def load_hs_chunk(c):
            tiles = []
            c0 = c * QB
            for k in range(KH):
                t = hsq.tile([128, QB], mdt, name=f"hsq_{c}_{k}", tag=f"hsq{k}",
                             bufs=2)
                nc.sync.dma_start(
                    t[:], bc(hsT.ap()[k * 128:(k + 1) * 128, c0:c0 + QB]))
                tiles.append(t)
            return tiles

        def load_wq(c, h):
            t = wqp.tile([128, HID], mdt, name=f"wq_{c}_{h}", tag="wq", bufs=2)
            # 4 parallel DMAs so the 1MB load spreads across queues
            for q in range(4):
                qs = q * (HID // 4)
                nc.sync.dma_start(
                    t[:, qs:qs + HID // 4],
                    bc(wqh.ap()[:, h * HID + qs:h * HID + qs + HID // 4]))
            return t

        # PE warmup: ~16 matmuls on a zeroed scratch tile run during the
        # initial DMA wait, so the HAM un-throttles before real work arrives.
        with (
            tc.tile_pool(name="warm", bufs=1) as warm,
            tc.tile_pool(name="warmp", bufs=1, space="PSUM") as warmp,
        ):
            wz = warm.tile([128, 512], mdt)
            nc.vector.memset(wz[:], 0.0)
            wps = [warmp.tile([128, 512], f32, name=f"wps{i}") for i in range(2)]
            for i in range(14):
                nc.tensor.matmul(wps[i % 2][:], wz[:, 0:128], wz[:],
                                 start=True, stop=True)

        # ---- Phase 1: K/V projections (feature-major) ----
        vTp = ctx.enter_context(tc.tile_pool(name="vTp", bufs=1))
        if True:
            vT_t = [vTp.tile([128, S], mdt, name=f"vT{f}") for f in range(KVH)]
            with (
                tc.tile_pool(name="st1", bufs=8) as st1,
                tc.tile_pool(name="pkv", bufs=1, space="PSUM") as pkv,
            ):
                for tb in range(NTB):
                    tb0 = tb * TBP
                    psk = [[pkv.tile([128, sl.stop - sl.start], f32,
                                     name=f"psk_{tb}_{f}_{j}", tag=f"psk{f}_{j}")
                            for j, sl in enumerate(NB5)] for f in range(KVH)]
                    psv = [[pkv.tile([128, sl.stop - sl.start], f32,
                                     name=f"psv_{tb}_{f}_{j}", tag=f"psv{f}_{j}")
                            for j, sl in enumerate(NB5)] for f in range(KVH)]
                    for k in range(KH):
                        hs_s = st1.tile([128, TBP], mdt, name=f"hs_{tb}_{k}", tag="hs")
                        nc.sync.dma_start(
                            hs_s[:], bc(hsT.ap()[k * 128:(k + 1) * 128, tb0:tb0 + TBP]))
                        wkv_s = st1.tile([128, 2 * FKV], mdt, name=f"wkv_{tb}_{k}",
                                         tag="wkv", bufs=8)
                        nc.sync.dma_start(wkv_s[:], bc(wkv.ap()[k * 128:(k + 1) * 128, :]))
                        wk_s = wkv_s[:, 0:FKV]
                        wv_s = wkv_s[:, FKV:2 * FKV]
                        if tb == NTB - 1 and k == KH // 2:
                            # prefetch chunk-0 inputs: queues drain phase A's
                            # remaining loads, then these land right at c0
                            wq_cur = load_wq(0, 0)
                            hs_c = load_hs_chunk(0)
                        for f in range(KVH):
                            for j, sl in enumerate(NB5):
                                nc.tensor.matmul(psk[f][j][:], wk_s[:, f * 128:(f + 1) * 128],
                                                 hs_s[:, sl], start=(k == 0), stop=(k == KH - 1))
                        for f in range(KVH):
                            for j, sl in enumerate(NB5):
                                nc.tensor.matmul(psv[f][j][:], wv_s[:, f * 128:(f + 1) * 128],
                                                 hs_s[:, sl], start=(k == 0), stop=(k == KH - 1))
                    if tb == 0:
                        nc.sync.dma_start(bq_t[:], bq.ap().rearrange("h p -> p h"))
                        nc.sync.dma_start(bk_t[:], bk.ap().rearrange("h p -> p h"))
                        nc.sync.dma_start(bv_t[:], bv.ap().rearrange("h p -> p h"))
                        nc.sync.dma_start(id_t[:], bc(ident.ap()))
                        nc.sync.dma_start(ones_t[:], bc(ones.ap()))
                        nc.sync.dma_start(cos_t[:], bc(cosT.ap()))
                        nc.sync.dma_start(sin_t[:], bc(sinT.ap()))
                    for f in range(KVH):
                        for j, sl in enumerate(NB5):
                            tsl = slice(tb0 + sl.start, tb0 + sl.stop)
                            nc.scalar.activation(k_t[f][:, tsl], psk[f][j][:], Ident,
                                                 bias=bk_t[:, f:f + 1])
                            nc.vector.tensor_scalar_add(vT_t[f][:, tsl], psv[f][j][:],
                                                        bv_t[:, f:f + 1])
                    # RoPE on this tb's K tokens (PSUM-free; overlaps the next pass)
                    for f in range(KVH):
                        rope_inplace(k_t[f], slice(tb0, tb0 + TBP), f"k{f}_{tb}")

        vv = v_t[:].rearrange("p (kt fkv) -> p kt fkv", fkv=FKV)
        GT = 4 if KT % 4 == 0 else 1
        tgroups = [(f, kt0) for f in range(KVH) for kt0 in range(0, KT, GT)]

        # ---- Phase 3+4+5: chunked Q-proj | attention | o_proj pipeline ----
        with (
            tc.tile_pool(name="expp", bufs=1) as expp,
            tc.tile_pool(name="spool", bufs=1) as spool,
            tc.tile_pool(name="invp", bufs=2) as invp,
            tc.tile_pool(name="pss", bufs=1, space="PSUM") as pss,
            tc.tile_pool(name="pso", bufs=1, space="PSUM") as pso,
            tc.tile_pool(name="aux", bufs=1, space="PSUM") as aux,
        ):
            def attn_begin(h, c):
                po = pso.tile([128, QB], f32, name=f"po_{h}_{c}", tag="oo", bufs=1)
                return {"po": po, "ranks": {}, "h": h, "c": c}

            def attn_group(st, g):
                h, c = st["h"], st["c"]
                f = h // (QH // KVH)
                sl = slice(c * QB, (c + 1) * QB)
                kt0 = 2 * g
                ps = pss.tile([128, 2 * QB], f32, name=f"ps_{h}_{c}_{g}", tag="ss",
                              bufs=2)
                for j in range(2):
                    nc.tensor.matmul(ps[:, j * QB:(j + 1) * QB],
                                     k_t[f][:, (kt0 + j) * 128:(kt0 + j + 1) * 128],
                                     q_t[h][:, sl], start=True, stop=True)
                et = expp.tile([128, 2 * QB], mdt, name=f"e_{h}_{c}_{g}", tag="et",
                               bufs=4)
                nc.scalar.activation(et[:], ps[:], Exp, scale=SCALE)
                for j in range(2):
                    kt = kt0 + j
                    nc.tensor.matmul(st["po"][:],
                                     v_t[:, kt * FKV + f * 128: kt * FKV + (f + 1) * 128],
                                     et[:, j * QB:(j + 1) * QB],
                                     start=(kt == 0), stop=(kt == KT - 1))
                node = spool.tile([128, QB], mdt, name=f"pa_{h}_{c}_{g}",
                                  tag=f"pa{g % 4}", bufs=2)
                nc.vector.tensor_add(node[:], et[:, 0:QB], et[:, QB:2 * QB])
                rank = 1
                ranks = st["ranks"]
                while rank in ranks:
                    prev = ranks.pop(rank)
                    nc.vector.tensor_add(prev[:], prev[:], node[:])
                    node, rank = prev, rank + 1
                ranks[rank] = node

            def attn_final(st):
                h, c = st["h"], st["c"]
                sl = slice(c * QB, (c + 1) * QB)
                rem = [st["ranks"][r] for r in sorted(st["ranks"])]
                ssum = rem[0]
                for other in rem[1:]:
                    nc.vector.tensor_add(ssum[:], ssum[:], other[:])
                pb = aux.tile([128, QB], f32, name=f"pb_{h}_{c}", tag="aux",
                              bufs=1)
                nc.tensor.matmul(pb[:], ones_t[:], ssum[:], start=True, stop=True)
                inv = invp.tile([128, QB], f32, name=f"inv_{h}_{c}", tag="inv")
                nc.vector.reciprocal_approx_fast(inv[:], pb[:])
                nc.vector.tensor_mul(q_t[h][:, sl], st["po"][:], inv[:])

            # -- chunks 0..NCH-1: Q-proj(c) interleaved with attention(c-1) --
            with tc.tile_pool(name="pq", bufs=1, space="PSUM") as pq:
                def transpose_group(gi):
                    # one V-transpose group (4 PE transposes + ScalarE copy)
                    f, kt0 = tgroups[gi]
                    pst = aux.tile([128, GT * 128], mdt, name=f"pst_{f}_{kt0}",
                                   tag="aux", bufs=1)
                    for j in range(GT):
                        nc.tensor.transpose(
                            pst[:, j * 128:(j + 1) * 128],
                            vT_t[f][:, (kt0 + j) * 128:(kt0 + j + 1) * 128], id_t[:])
                    nc.vector.tensor_copy(
                        vv[:, kt0:kt0 + GT, f * 128:(f + 1) * 128],
                        pst[:].rearrange("p (j c) -> p j c", c=128))

                tg_iter = iter(range(len(tgroups)))

                def tg_next():
                    gi = next(tg_iter, None)
                    if gi is not None:
                        transpose_group(gi)

                for c in range(NCH):
                    csl = slice(c * QB, (c + 1) * QB)
                    for h in range(QH):
                        # prefetch next head's weights (or next chunk's head 0)
                        if h + 1 < QH:
                            wq_nxt = load_wq(c, h + 1)
                        elif c + 1 < NCH:
                            wq_nxt = load_wq(c + 1, 0)
                        else:
                            wq_nxt = None
                        psq_t = pq.tile([128, QB], f32, name=f"psq_{c}_{h}",
                                        tag="psq", bufs=2)
                        if c == 0:
                            # V-transposes ride inside chunk 0; front-load two
                            # groups as filler while the prefetch DMAs land
                            if h == 0:
                                tg_next()
                                tg_next()
                            for k in range(KH // 2):
                                nc.tensor.matmul(psq_t[:],
                                                 wq_cur[:, k * 128:(k + 1) * 128],
                                                 hs_c[k][:], start=(k == 0),
                                                 stop=(k == KH - 1))
                            tg_next()
                            for k in range(KH // 2, KH):
                                nc.tensor.matmul(psq_t[:],
                                                 wq_cur[:, k * 128:(k + 1) * 128],
                                                 hs_c[k][:], start=(k == 0),
                                                 stop=(k == KH - 1))
                        else:
                            st = attn_begin(h, c - 1)
                            for g in range(NG):
                                attn_group(st, g)
                                for k in range(4 * g, 4 * g + 4):
                                    nc.tensor.matmul(psq_t[:],
                                                     wq_cur[:, k * 128:(k + 1) * 128],
                                                     hs_c[k][:], start=(k == 0),
                                                     stop=(k == KH - 1))
                            attn_final(st)
                        if h == QH // 2 and c + 1 < NCH:
                            nxt_hs = load_hs_chunk(c + 1)
                        nc.vector.tensor_scalar_add(q_t[h][:, csl], psq_t[:],
                                                    bq_t[:, h:h + 1])
                        rope_inplace(q_t[h], csl, f"q{h}_{c}")
                        wq_cur = wq_nxt
                    if c + 1 < NCH:
                        hs_c = nxt_hs

            # -- final chunk's attention interleaved with o_proj --
            with (
                tc.tile_pool(name="st3", bufs=4) as st3,
                tc.tile_pool(name="osb", bufs=2) as osb,
                tc.tile_pool(name="po5", bufs=1, space="PSUM") as po5,
            ):
                NHB = HID // 512
                NT1 = KT - QB // 128  # token tiles in chunks 0..NCH-2: 12

                def load_woh(i, seq):
                    if i >= len(seq):
                        return None
                    phase, hb = seq[i]
                    w = st3.tile([128, QH * 512], mdt,
                                 name=f"wo_{phase}_{hb}", tag="woh", bufs=2)
                    for q in range(4):
                        qs = q * (QH * 512 // 4)
                        nc.sync.dma_start(
                            w[:, qs:qs + QH * 512 // 4],
                            bc(woh.ap()[:, hb * QH * 512 + qs:
                                        hb * QH * 512 + qs + QH * 512 // 4]))
                    return w

                def oproj_units():
                    seq = [(p, hb) for p in (0, 1) for hb in range(NHB)]
                    tiles = {0: load_woh(0, seq), 1: load_woh(1, seq)}
                    for i, (phase, hb) in enumerate(seq):
                        w = tiles.pop(i)
                        tts = range(NT1) if phase == 0 else range(NT1, KT)
                        for ti, tt in enumerate(tts):
                            if ti == 1 and i + 2 not in tiles:
                                tiles[i + 2] = load_woh(i + 2, seq)
                            pot = po5.tile([128, 512], f32, name=f"pot_{hb}_{tt}",
                                           tag="po", bufs=2)
                            for fh in range(QH):
                                nc.tensor.matmul(pot[:],
                                                 q_t[fh][:, tt * 128:(tt + 1) * 128],
                                                 w[:, fh * 512:(fh + 1) * 512],
                                                 start=(fh == 0), stop=(fh == QH - 1))
                            ot = osb.tile([128, 512], f32, name=f"ot_{hb}_{tt}",
                                          tag="ot", bufs=4)
                            nc.vector.tensor_copy(ot[:], pot[:])
                            nc.sync.dma_start(
                                out.ap()[tt * 128:(tt + 1) * 128,
                                         hb * 512:(hb + 1) * 512], ot[:])
                            yield

                gen = oproj_units()
                g_idx = 0
                for h in range(QH):
                    st = attn_begin(h, NCH - 1)
                    for g in range(NG):
                        attn_group(st, g)
                        next(gen)
                        if g_idx % 2 == 0:
                            next(gen)   # 96 pass-1 units over 64 groups
                        g_idx += 1
                    attn_final(st)
                for _ in gen:
                    pass

    nc.compile()
    return nc


def make_host_constants():
    ident = np.eye(128, dtype=np.float32)
    ones = np.ones((128, 128), dtype=np.float32)
    return ident, ones


def shard_inputs(hidden_states, cos, sin, Wq, bq, Wk, bk, Wv, bv, Wo, S=S_FULL,
                 dt="bf16"):
    ident, ones = make_host_constants()
    if dt == "bf16":
        import ml_dtypes
        big = ml_dtypes.bfloat16
    else:
        big = np.float32
    in_maps = []
    for c in range(8):
        b, t = c // TP, c % TP
        sinT = np.ascontiguousarray(sin[b].T).astype(np.float32)
        sinT[:HD // 2, :] *= -1.0   # rotate_half sign folded into the table
        wq_slice = Wq[:, t * FQ:(t + 1) * FQ]
        # [p, h*HID + kt*128 + cc] layout: one contiguous DMA per head
        wqh = np.ascontiguousarray(
            wq_slice.reshape(KH, 128, QH, HD).transpose(1, 2, 0, 3).reshape(
                128, QH * HID))
        wo_slice = Wo[t * FQ:(t + 1) * FQ, :]
        # [p, hb*QH*512 + fh*512 + c] layout: one contiguous DMA per hid block
        woh = np.ascontiguousarray(
            wo_slice.reshape(QH, 128, HID // 512, 512).transpose(1, 2, 0, 3)
            .reshape(128, QH * HID))
        m = {
            "hsT": np.ascontiguousarray(hidden_states[b].T).astype(big),
            "cosT": np.ascontiguousarray(cos[b].T).astype(big),
            "sinT": sinT.astype(big),
            "wqh": wqh.astype(big),
            "bq": np.ascontiguousarray(bq[t * FQ:(t + 1) * FQ].reshape(QH, HD)),
            "bk": np.ascontiguousarray(bk[t * FKV:(t + 1) * FKV].reshape(KVH, HD)),
            "bv": np.ascontiguousarray(bv[t * FKV:(t + 1) * FKV].reshape(KVH, HD)),
            "wkv": np.ascontiguousarray(np.concatenate(
                [Wk[:, t * FKV:(t + 1) * FKV], Wv[:, t * FKV:(t + 1) * FKV]],
                axis=1)).astype(big),
            "woh": woh.astype(big),
            "ident": ident.astype(big), "ones": ones.astype(big),
        }
        in_maps.append(m)
    return in_maps


_nc_cache = {}


def kernel(hidden_states, cos, sin, Wq, bq, Wk, bk, Wv, bv, Wo):
    global last_exec_time_ns
    from concourse.bass_utils import run_bass_kernel_spmd

    hidden_states = np.asarray(hidden_states, dtype=np.float32)
    cos = np.asarray(cos, dtype=np.float32)
    sin = np.asarray(sin, dtype=np.float32)
    S = hidden_states.shape[1]
    dt = os.environ.get("ATTN_DT", "bf16")
    if (S, dt) not in _nc_cache:
        _nc_cache[(S, dt)] = build_nc(S, dt)
    nc = _nc_cache[(S, dt)]
    in_maps = shard_inputs(hidden_states, cos, sin,
                           np.asarray(Wq, np.float32), np.asarray(bq, np.float32),
                           np.asarray(Wk, np.float32), np.asarray(bk, np.float32),
                           np.asarray(Wv, np.float32), np.asarray(bv, np.float32),
                           np.asarray(Wo, np.float32), S=S, dt=dt)
    trace = bool(int(os.environ.get("ATTN_TRACE", "0")))
    r = run_bass_kernel_spmd(nc, in_maps, list(range(8)), trace=trace)
    last_exec_time_ns = r.exec_time_ns
    outs = [r.results[c]["out"] for c in range(8)]
    full = np.empty((B, S, HID), dtype=np.float32)
    for b in range(B):
        full[b] = outs[b * TP]
        for t in range(1, TP):
            full[b] += outs[b * TP + t]
    return full


# revision 33
# speedup vs baseline: 1.0081x; 1.0081x over previous
"""Trainium2 Bass kernel for GQA attention block (B=2, S=2048, HID=4096, 32Q/8KV heads).

Sharding: hybrid TP4 x DP2 over 8 NeuronCores.
  core c: batch b = c // 4, TP slice t = c % 4.
  Each core handles one batch element, 8 Q heads (2 KV heads); o_proj partials
  summed on host.

On-chip dataflow is feature-major: qT/kT [head_dim, tokens]; scores come out
keys-major (softmax key-sum via ones-matmul), attention output lands as o_proj
lhsT. RoPE = two SBUF partition-shift DMAs + vector combine (sign folded into
host-negated sin table).

Software pipeline (v2): attention's softmax exp saturates ScalarE (~285us of
EXP for the whole kernel), so Q-projection is interleaved with attention at
score-group granularity: token chunk c's Q-proj matmuls fill the PE while
chunk c-1's attention exp runs on ScalarE. The final chunk's attention
interleaves with o_proj tiles. All Q-evac/rope work rides on VectorE so
ScalarE does exp only. PSUM: pss(4) + po(1) + psb(1) + psq|pot(2) = 8 banks.
"""
import os
import sys

for _p in ("/opt/trn_rl_repo", "/root/.axon_site"):
    if _p not in sys.path and os.path.isdir(_p):
        sys.path.append(_p)

import numpy as np

B, S_FULL, HID = 2, 2048, 4096
NH, NKV, HD = 32, 8, 128
TP = 4                 # tensor-parallel ways
QH = NH // TP          # 8 q heads per core
KVH = NKV // TP        # 2 kv heads per core
FQ = QH * HD           # 1024
FKV = KVH * HD         # 256
KH = HID // 128        # 32 contraction tiles
SCALE = 1.0 / float(np.sqrt(HD))

last_exec_time_ns = None


def build_nc(S: int = S_FULL, dt: str = "bf16"):
    """Build the per-core Bass program (SPMD: same program, per-core inputs)."""
    import concourse.bass as bass
    import concourse.tile as tile
    from concourse import bacc, mybir
    from contextlib import ExitStack

    f32 = mybir.dt.float32
    mdt = mybir.dt.bfloat16 if dt == "bf16" else mybir.dt.float32r
    ddt = mdt if dt == "bf16" else f32   # DRAM dtype for big inputs
    TBP = min(1024, S)                   # kv projection token block
    NTB = S // TBP
    NB5 = [slice(j * 512, min((j + 1) * 512, TBP)) for j in range((TBP + 511) // 512)]
    KT = S // 128                        # key/token tiles
    HALF = HD // 2
    QB = min(512, S)                     # pipeline token chunk
    NCH = S // QB                        # 4 chunks
    NG = KT // 2                         # score groups per (head, chunk): 8

    nc = bacc.Bacc("TRN2", target_bir_lowering=False, debug=False)

    hsT = nc.dram_tensor("hsT", [HID, S], ddt, kind="ExternalInput")
    cosT = nc.dram_tensor("cosT", [HD, S], ddt, kind="ExternalInput")
    sinT = nc.dram_tensor("sinT", [HD, S], ddt, kind="ExternalInput")  # sign-folded
    wqh = nc.dram_tensor("wqh", [128, QH * HID], ddt, kind="ExternalInput")
    bq = nc.dram_tensor("bq", [QH, HD], f32, kind="ExternalInput")
    wkv = nc.dram_tensor("wkv", [HID, 2 * FKV], ddt, kind="ExternalInput")
    bk = nc.dram_tensor("bk", [KVH, HD], f32, kind="ExternalInput")
    bv = nc.dram_tensor("bv", [KVH, HD], f32, kind="ExternalInput")
    woh = nc.dram_tensor("woh", [128, (HID // 512) * QH * 512], ddt,
                         kind="ExternalInput")
    ident = nc.dram_tensor("ident", [128, 128], ddt, kind="ExternalInput")
    ones = nc.dram_tensor("ones", [128, 128], ddt, kind="ExternalInput")
    out = nc.dram_tensor("out", [S, HID], mdt, kind="ExternalOutput")

    def bc(ap):
        """View a DRAM fp32 AP as f32r (no-op for bf16)."""
        return ap.bitcast(mdt) if dt == "f32r" else ap

    with tile.TileContext(nc) as tc, ExitStack() as ctx:
        Exp = mybir.ActivationFunctionType.Exp
        Ident = mybir.ActivationFunctionType.Identity

        const = ctx.enter_context(tc.tile_pool(name="const", bufs=1))
        bq_t = const.tile([128, QH], f32)
        bk_t = const.tile([128, KVH], f32)
        bv_t = const.tile([128, KVH], f32)
        id_t = const.tile([128, 128], mdt)
        ones_t = const.tile([128, 128], mdt)
        cos_t = const.tile([128, S], mdt)
        sin_t = const.tile([128, S], mdt)

        # Persistent activations (feature-major). attnT overwrites q in place.
        qpool = ctx.enter_context(tc.tile_pool(name="qpool", bufs=1))
        q_t = [qpool.tile([128, S], mdt, name=f"q{h}") for h in range(QH)]
        kvpool = ctx.enter_context(tc.tile_pool(name="kvpool", bufs=1))
        k_t = [kvpool.tile([128, S], mdt, name=f"k{f}") for f in range(KVH)]
        v_t = kvpool.tile([128, KT * FKV], mdt, name="v")  # [tok%128, (kt, kv*128+d)]

        rope_pool = ctx.enter_context(tc.tile_pool(name="ropep", bufs=1))

        def rope_inplace(x_t, sl, tag):
            """x[:, sl] = x[:, sl]*cos + shift64(x[:, sl])*sin', VectorE + DMA only."""
            w = sl.stop - sl.start
            rot = rope_pool.tile([128, 1024], mdt, name=f"rot_{tag}", tag="rot", bufs=4)
            nc.sync.dma_start(rot[0:HALF, :w], x_t[HALF:128, sl])
            nc.sync.dma_start(rot[HALF:128, :w], x_t[0:HALF, sl])
            t1 = rope_pool.tile([128, 1024], mdt, name=f"t1_{tag}", tag="t1", bufs=4)
            nc.vector.tensor_mul(t1[:, :w], rot[:, :w], sin_t[:, sl])
            nc.vector.tensor_mul(x_t[:, sl], x_t[:, sl], cos_t[:, sl])
            nc.vector.tensor_add(x_t[:, sl], x_t[:, sl], t1[:, :w])

        # Pipeline input pools opened early so chunk-0 prefetch can overlap
        # the V-transpose section.
        hsq = ctx.enter_context(tc.tile_pool(name="hsq", bufs=1))
        wqp = ctx.enter_context(tc.tile_pool(name="wqp", bufs=1))
        QB = min(512, S)
        NCH = S // QB
        NG = KT // 2

        def load_hs_chunk(c):
            tiles = []
            c0 = c * QB
            for k in range(KH):
                t = hsq.tile([128, QB], mdt, name=f"hsq_{c}_{k}", tag=f"hsq{k}",
                             bufs=2)
                nc.sync.dma_start(
                    t[:], bc(hsT.ap()[k * 128:(k + 1) * 128, c0:c0 + QB]))
                tiles.append(t)
            return tiles

        def load_wq(c, h):
            t = wqp.tile([128, HID], mdt, name=f"wq_{c}_{h}", tag="wq", bufs=2)
            # 4 parallel DMAs so the 1MB load spreads across queues
            for q in range(4):
                qs = q * (HID // 4)
                nc.sync.dma_start(
                    t[:, qs:qs + HID // 4],
                    bc(wqh.ap()[:, h * HID + qs:h * HID + qs + HID // 4]))
            return t

        # PE warmup: ~16 matmuls on a zeroed scratch tile run during the
        # initial DMA wait, so the HAM un-throttles before real work arrives.
        with (
            tc.tile_pool(name="warm", bufs=1) as warm,
            tc.tile_pool(name="warmp", bufs=1, space="PSUM") as warmp,
        ):
            wz = warm.tile([128, 512], mdt)
            nc.vector.memset(wz[:], 0.0)
            wps = [warmp.tile([128, 512], f32, name=f"wps{i}") for i in range(2)]
            for i in range(14):
                nc.tensor.matmul(wps[i % 2][:], wz[:, 0:128], wz[:],
                                 start=True, stop=True)

        # ---- Phase 1: K/V projections (feature-major) ----
        vTp = ctx.enter_context(tc.tile_pool(name="vTp", bufs=1))
        if True:
            vT_t = [vTp.tile([128, S], mdt, name=f"vT{f}") for f in range(KVH)]
            with (
                tc.tile_pool(name="st1", bufs=6) as st1,
                tc.tile_pool(name="wkvp", bufs=1) as wkvp,
                tc.tile_pool(name="pkv", bufs=1, space="PSUM") as pkv,
            ):
                wkv_t = [None] * KH
                for tb in range(NTB):
                    tb0 = tb * TBP
                    psk = [[pkv.tile([128, sl.stop - sl.start], f32,
                                     name=f"psk_{tb}_{f}_{j}", tag=f"psk{f}_{j}")
                            for j, sl in enumerate(NB5)] for f in range(KVH)]
                    psv = [[pkv.tile([128, sl.stop - sl.start], f32,
                                     name=f"psv_{tb}_{f}_{j}", tag=f"psv{f}_{j}")
                            for j, sl in enumerate(NB5)] for f in range(KVH)]
                    for k in range(KH):
                        hs_s = st1.tile([128, TBP], mdt, name=f"hs_{tb}_{k}", tag="hs")
                        nc.sync.dma_start(
                            hs_s[:], bc(hsT.ap()[k * 128:(k + 1) * 128, tb0:tb0 + TBP]))
                        if tb == 0:
                            wkv_t[k] = wkvp.tile([128, 2 * FKV], mdt,
                                                 name=f"wkv_{k}")
                            nc.sync.dma_start(wkv_t[k][:],
                                              bc(wkv.ap()[k * 128:(k + 1) * 128, :]))
                        wk_s = wkv_t[k][:, 0:FKV]
                        wv_s = wkv_t[k][:, FKV:2 * FKV]
                        for f in range(KVH):
                            for j, sl in enumerate(NB5):
                                nc.tensor.matmul(psk[f][j][:], wk_s[:, f * 128:(f + 1) * 128],
                                                 hs_s[:, sl], start=(k == 0), stop=(k == KH - 1))
                        for f in range(KVH):
                            for j, sl in enumerate(NB5):
                                nc.tensor.matmul(psv[f][j][:], wv_s[:, f * 128:(f + 1) * 128],
                                                 hs_s[:, sl], start=(k == 0), stop=(k == KH - 1))
                    if tb == 0:
                        nc.sync.dma_start(bq_t[:], bq.ap().rearrange("h p -> p h"))
                        nc.sync.dma_start(bk_t[:], bk.ap().rearrange("h p -> p h"))
                        nc.sync.dma_start(bv_t[:], bv.ap().rearrange("h p -> p h"))
                        nc.sync.dma_start(id_t[:], bc(ident.ap()))
                        nc.sync.dma_start(ones_t[:], bc(ones.ap()))
                        nc.sync.dma_start(cos_t[:], bc(cosT.ap()))
                        nc.sync.dma_start(sin_t[:], bc(sinT.ap()))
                    for f in range(KVH):
                        for j, sl in enumerate(NB5):
                            tsl = slice(tb0 + sl.start, tb0 + sl.stop)
                            nc.scalar.activation(k_t[f][:, tsl], psk[f][j][:], Ident,
                                                 bias=bk_t[:, f:f + 1])
                            nc.vector.tensor_scalar_add(vT_t[f][:, tsl], psv[f][j][:],
                                                        bv_t[:, f:f + 1])
                    # RoPE on this tb's K tokens (PSUM-free; overlaps the next pass)
                    for f in range(KVH):
                        rope_inplace(k_t[f], slice(tb0, tb0 + TBP), f"k{f}_{tb}")

            # Prefetch chunk-0 inputs; lands during phase A's PE tail.
            wq_cur = load_wq(0, 0)
            hs_c = load_hs_chunk(0)

        vv = v_t[:].rearrange("p (kt fkv) -> p kt fkv", fkv=FKV)
        GT = 4 if KT % 4 == 0 else 1
        tgroups = [(f, kt0) for f in range(KVH) for kt0 in range(0, KT, GT)]

        # ---- Phase 3+4+5: chunked Q-proj | attention | o_proj pipeline ----
        with (
            tc.tile_pool(name="expp", bufs=1) as expp,
            tc.tile_pool(name="spool", bufs=1) as spool,
            tc.tile_pool(name="invp", bufs=2) as invp,
            tc.tile_pool(name="pss", bufs=1, space="PSUM") as pss,
            tc.tile_pool(name="pso", bufs=1, space="PSUM") as pso,
            tc.tile_pool(name="aux", bufs=1, space="PSUM") as aux,
        ):
            def attn_begin(h, c):
                po = pso.tile([128, QB], f32, name=f"po_{h}_{c}", tag="oo", bufs=1)
                return {"po": po, "ranks": {}, "h": h, "c": c}

            def attn_group(st, g):
                h, c = st["h"], st["c"]
                f = h // (QH // KVH)
                sl = slice(c * QB, (c + 1) * QB)
                kt0 = 2 * g
                ps = pss.tile([128, 2 * QB], f32, name=f"ps_{h}_{c}_{g}", tag="ss",
                              bufs=2)
                for j in range(2):
                    nc.tensor.matmul(ps[:, j * QB:(j + 1) * QB],
                                     k_t[f][:, (kt0 + j) * 128:(kt0 + j + 1) * 128],
                                     q_t[h][:, sl], start=True, stop=True)
                et = expp.tile([128, 2 * QB], mdt, name=f"e_{h}_{c}_{g}", tag="et",
                               bufs=4)
                nc.scalar.activation(et[:], ps[:], Exp, scale=SCALE)
                for j in range(2):
                    kt = kt0 + j
                    nc.tensor.matmul(st["po"][:],
                                     v_t[:, kt * FKV + f * 128: kt * FKV + (f + 1) * 128],
                                     et[:, j * QB:(j + 1) * QB],
                                     start=(kt == 0), stop=(kt == KT - 1))
                node = spool.tile([128, QB], mdt, name=f"pa_{h}_{c}_{g}",
                                  tag=f"pa{g % 4}", bufs=2)
                nc.vector.tensor_add(node[:], et[:, 0:QB], et[:, QB:2 * QB])
                rank = 1
                ranks = st["ranks"]
                while rank in ranks:
                    prev = ranks.pop(rank)
                    nc.vector.tensor_add(prev[:], prev[:], node[:])
                    node, rank = prev, rank + 1
                ranks[rank] = node

            def attn_final(st):
                h, c = st["h"], st["c"]
                sl = slice(c * QB, (c + 1) * QB)
                rem = [st["ranks"][r] for r in sorted(st["ranks"])]
                ssum = rem[0]
                for other in rem[1:]:
                    nc.vector.tensor_add(ssum[:], ssum[:], other[:])
                pb = aux.tile([128, QB], f32, name=f"pb_{h}_{c}", tag="aux",
                              bufs=1)
                nc.tensor.matmul(pb[:], ones_t[:], ssum[:], start=True, stop=True)
                inv = invp.tile([128, QB], f32, name=f"inv_{h}_{c}", tag="inv")
                nc.vector.reciprocal_approx_fast(inv[:], pb[:])
                nc.vector.tensor_mul(q_t[h][:, sl], st["po"][:], inv[:])

            # -- chunks 0..NCH-1: Q-proj(c) interleaved with attention(c-1) --
            with tc.tile_pool(name="pq", bufs=1, space="PSUM") as pq:
                def transpose_group(gi):
                    # one V-transpose group (4 PE transposes + ScalarE copy)
                    f, kt0 = tgroups[gi]
                    pst = aux.tile([128, GT * 128], mdt, name=f"pst_{f}_{kt0}",
                                   tag="aux", bufs=1)
                    for j in range(GT):
                        nc.tensor.transpose(
                            pst[:, j * 128:(j + 1) * 128],
                            vT_t[f][:, (kt0 + j) * 128:(kt0 + j + 1) * 128], id_t[:])
                    nc.vector.tensor_copy(
                        vv[:, kt0:kt0 + GT, f * 128:(f + 1) * 128],
                        pst[:].rearrange("p (j c) -> p j c", c=128))

                tg_iter = iter(range(len(tgroups)))

                def tg_next():
                    gi = next(tg_iter, None)
                    if gi is not None:
                        transpose_group(gi)

                for c in range(NCH):
                    csl = slice(c * QB, (c + 1) * QB)
                    for h in range(QH):
                        # prefetch next head's weights (or next chunk's head 0)
                        if h + 1 < QH:
                            wq_nxt = load_wq(c, h + 1)
                        elif c + 1 < NCH:
                            wq_nxt = load_wq(c + 1, 0)
                        else:
                            wq_nxt = None
                        psq_t = pq.tile([128, QB], f32, name=f"psq_{c}_{h}",
                                        tag="psq", bufs=2)
                        if c == 0:
                            # V-transposes ride inside chunk 0; front-load two
                            # groups as filler while the prefetch DMAs land
                            if h == 0:
                                tg_next()
                                tg_next()
                            for k in range(KH // 2):
                                nc.tensor.matmul(psq_t[:],
                                                 wq_cur[:, k * 128:(k + 1) * 128],
                                                 hs_c[k][:], start=(k == 0),
                                                 stop=(k == KH - 1))
                            tg_next()
                            for k in range(KH // 2, KH):
                                nc.tensor.matmul(psq_t[:],
                                                 wq_cur[:, k * 128:(k + 1) * 128],
                                                 hs_c[k][:], start=(k == 0),
                                                 stop=(k == KH - 1))
                        else:
                            st = attn_begin(h, c - 1)
                            for g in range(NG):
                                attn_group(st, g)
                                for k in range(4 * g, 4 * g + 4):
                                    nc.tensor.matmul(psq_t[:],
                                                     wq_cur[:, k * 128:(k + 1) * 128],
                                                     hs_c[k][:], start=(k == 0),
                                                     stop=(k == KH - 1))
                            attn_final(st)
                        if h == QH // 2 and c + 1 < NCH:
                            nxt_hs = load_hs_chunk(c + 1)
                        nc.vector.tensor_scalar_add(q_t[h][:, csl], psq_t[:],
                                                    bq_t[:, h:h + 1])
                        rope_inplace(q_t[h], csl, f"q{h}_{c}")
                        wq_cur = wq_nxt
                    if c + 1 < NCH:
                        hs_c = nxt_hs

            # -- final chunk's attention interleaved with o_proj --
            with (
                tc.tile_pool(name="st3", bufs=4) as st3,
                tc.tile_pool(name="osb", bufs=2) as osb,
                tc.tile_pool(name="po5", bufs=1, space="PSUM") as po5,
            ):
                NHB = HID // 512
                NT1 = KT - QB // 128  # token tiles in chunks 0..NCH-2: 12

                def load_woh(i, seq):
                    if i >= len(seq):
                        return None
                    phase, hb = seq[i]
                    w = st3.tile([128, QH * 512], mdt,
                                 name=f"wo_{phase}_{hb}", tag="woh", bufs=2)
                    for q in range(4):
                        qs = q * (QH * 512 // 4)
                        nc.sync.dma_start(
                            w[:, qs:qs + QH * 512 // 4],
                            bc(woh.ap()[:, hb * QH * 512 + qs:
                                        hb * QH * 512 + qs + QH * 512 // 4]))
                    return w

                def oproj_units():
                    seq = [(p, hb) for p in (0, 1) for hb in range(NHB)]
                    tiles = {0: load_woh(0, seq), 1: load_woh(1, seq)}
                    for i, (phase, hb) in enumerate(seq):
                        w = tiles.pop(i)
                        tts = range(NT1) if phase == 0 else range(NT1, KT)
                        for ti, tt in enumerate(tts):
                            if ti == 1 and i + 2 not in tiles:
                                tiles[i + 2] = load_woh(i + 2, seq)
                            pot = po5.tile([128, 512], f32, name=f"pot_{hb}_{tt}",
                                           tag="po", bufs=2)
                            for fh in range(QH):
                                nc.tensor.matmul(pot[:],
                                                 q_t[fh][:, tt * 128:(tt + 1) * 128],
                                                 w[:, fh * 512:(fh + 1) * 512],
                                                 start=(fh == 0), stop=(fh == QH - 1))
                            ot = osb.tile([128, 512], mdt, name=f"ot_{hb}_{tt}",
                                          tag="ot", bufs=4)
                            nc.vector.tensor_copy(ot[:], pot[:])
                            nc.sync.dma_start(
                                out.ap()[tt * 128:(tt + 1) * 128,
                                         hb * 512:(hb + 1) * 512], ot[:])
                            yield

                gen = oproj_units()
                g_idx = 0
                for h in range(QH):
                    st = attn_begin(h, NCH - 1)
                    for g in range(NG):
                        attn_group(st, g)
                        next(gen)
                        if g_idx % 2 == 0:
                            next(gen)   # 96 pass-1 units over 64 groups
                        g_idx += 1
                    attn_final(st)
                for _ in gen:
                    pass

    nc.compile()
    return nc


def make_host_constants():
    ident = np.eye(128, dtype=np.float32)
    ones = np.ones((128, 128), dtype=np.float32)
    return ident, ones


def shard_inputs(hidden_states, cos, sin, Wq, bq, Wk, bk, Wv, bv, Wo, S=S_FULL,
                 dt="bf16"):
    ident, ones = make_host_constants()
    if dt == "bf16":
        import ml_dtypes
        big = ml_dtypes.bfloat16
    else:
        big = np.float32
    in_maps = []
    for c in range(8):
        b, t = c // TP, c % TP
        sinT = np.ascontiguousarray(sin[b].T).astype(np.float32)
        sinT[:HD // 2, :] *= -1.0   # rotate_half sign folded into the table
        wq_slice = Wq[:, t * FQ:(t + 1) * FQ]
        # [p, h*HID + kt*128 + cc] layout: one contiguous DMA per head
        wqh = np.ascontiguousarray(
            wq_slice.reshape(KH, 128, QH, HD).transpose(1, 2, 0, 3).reshape(
                128, QH * HID))
        wo_slice = Wo[t * FQ:(t + 1) * FQ, :]
        # [p, hb*QH*512 + fh*512 + c] layout: one contiguous DMA per hid block
        woh = np.ascontiguousarray(
            wo_slice.reshape(QH, 128, HID // 512, 512).transpose(1, 2, 0, 3)
            .reshape(128, QH * HID))
        m = {
            "hsT": np.ascontiguousarray(hidden_states[b].T).astype(big),
            "cosT": np.ascontiguousarray(cos[b].T).astype(big),
            "sinT": sinT.astype(big),
            "wqh": wqh.astype(big),
            "bq": np.ascontiguousarray(bq[t * FQ:(t + 1) * FQ].reshape(QH, HD)),
            "bk": np.ascontiguousarray(bk[t * FKV:(t + 1) * FKV].reshape(KVH, HD)),
            "bv": np.ascontiguousarray(bv[t * FKV:(t + 1) * FKV].reshape(KVH, HD)),
            "wkv": np.ascontiguousarray(np.concatenate(
                [Wk[:, t * FKV:(t + 1) * FKV], Wv[:, t * FKV:(t + 1) * FKV]],
                axis=1)).astype(big),
            "woh": woh.astype(big),
            "ident": ident.astype(big), "ones": ones.astype(big),
        }
        in_maps.append(m)
    return in_maps


_nc_cache = {}


def kernel(hidden_states, cos, sin, Wq, bq, Wk, bk, Wv, bv, Wo):
    global last_exec_time_ns
    from concourse.bass_utils import run_bass_kernel_spmd

    hidden_states = np.asarray(hidden_states, dtype=np.float32)
    cos = np.asarray(cos, dtype=np.float32)
    sin = np.asarray(sin, dtype=np.float32)
    S = hidden_states.shape[1]
    dt = os.environ.get("ATTN_DT", "bf16")
    if (S, dt) not in _nc_cache:
        _nc_cache[(S, dt)] = build_nc(S, dt)
    nc = _nc_cache[(S, dt)]
    in_maps = shard_inputs(hidden_states, cos, sin,
                           np.asarray(Wq, np.float32), np.asarray(bq, np.float32),
                           np.asarray(Wk, np.float32), np.asarray(bk, np.float32),
                           np.asarray(Wv, np.float32), np.asarray(bv, np.float32),
                           np.asarray(Wo, np.float32), S=S, dt=dt)
    trace = bool(int(os.environ.get("ATTN_TRACE", "0")))
    r = run_bass_kernel_spmd(nc, in_maps, list(range(8)), trace=trace)
    last_exec_time_ns = r.exec_time_ns
    outs = [np.asarray(r.results[c]["out"], dtype=np.float32) for c in range(8)]
    full = np.empty((B, S, HID), dtype=np.float32)
    for b in range(B):
        full[b] = outs[b * TP]
        for t in range(1, TP):
            full[b] += outs[b * TP + t]
    return full


# revision 36
# speedup vs baseline: 1.0097x; 1.0015x over previous
"""Trainium2 Bass kernel for GQA attention block (B=2, S=2048, HID=4096, 32Q/8KV heads).

Sharding: hybrid TP4 x DP2 over 8 NeuronCores.
  core c: batch b = c // 4, TP slice t = c % 4.
  Each core handles one batch element, 8 Q heads (2 KV heads); o_proj partials
  summed on host.

On-chip dataflow is feature-major: qT/kT [head_dim, tokens]; scores come out
keys-major (softmax key-sum via ones-matmul), attention output lands as o_proj
lhsT. RoPE = two SBUF partition-shift DMAs + vector combine (sign folded into
host-negated sin table).

Software pipeline (v2): attention's softmax exp saturates ScalarE (~285us of
EXP for the whole kernel), so Q-projection is interleaved with attention at
score-group granularity: token chunk c's Q-proj matmuls fill the PE while
chunk c-1's attention exp runs on ScalarE. The final chunk's attention
interleaves with o_proj tiles. All Q-evac/rope work rides on VectorE so
ScalarE does exp only. PSUM: pss(4) + po(1) + psb(1) + psq|pot(2) = 8 banks.
"""
import os
import sys

for _p in ("/opt/trn_rl_repo", "/root/.axon_site"):
    if _p not in sys.path and os.path.isdir(_p):
        sys.path.append(_p)

import numpy as np

B, S_FULL, HID = 2, 2048, 4096
NH, NKV, HD = 32, 8, 128
TP = 4                 # tensor-parallel ways
QH = NH // TP          # 8 q heads per core
KVH = NKV // TP        # 2 kv heads per core
FQ = QH * HD           # 1024
FKV = KVH * HD         # 256
KH = HID // 128        # 32 contraction tiles
SCALE = 1.0 / float(np.sqrt(HD))

last_exec_time_ns = None


def build_nc(S: int = S_FULL, dt: str = "bf16"):
    """Build the per-core Bass program (SPMD: same program, per-core inputs)."""
    import concourse.bass as bass
    import concourse.tile as tile
    from concourse import bacc, mybir
    from contextlib import ExitStack

    f32 = mybir.dt.float32
    mdt = mybir.dt.bfloat16 if dt == "bf16" else mybir.dt.float32r
    ddt = mdt if dt == "bf16" else f32   # DRAM dtype for big inputs
    TBP = min(1024, S)                   # kv projection token block
    NTB = S // TBP
    NB5 = [slice(j * 512, min((j + 1) * 512, TBP)) for j in range((TBP + 511) // 512)]
    KT = S // 128                        # key/token tiles
    HALF = HD // 2
    QB = min(512, S)                     # pipeline token chunk
    NCH = S // QB                        # 4 chunks
    NG = KT // 2                         # score groups per (head, chunk): 8

    nc = bacc.Bacc("TRN2", target_bir_lowering=False, debug=False)

    hsT = nc.dram_tensor("hsT", [HID, S], ddt, kind="ExternalInput")
    cosT = nc.dram_tensor("cosT", [HD, S], ddt, kind="ExternalInput")
    sinT = nc.dram_tensor("sinT", [HD, S], ddt, kind="ExternalInput")  # sign-folded
    wqh = nc.dram_tensor("wqh", [128, QH * HID], ddt, kind="ExternalInput")
    bq = nc.dram_tensor("bq", [QH, HD], f32, kind="ExternalInput")
    wkv = nc.dram_tensor("wkv", [HID, 2 * FKV], ddt, kind="ExternalInput")
    bk = nc.dram_tensor("bk", [KVH, HD], f32, kind="ExternalInput")
    bv = nc.dram_tensor("bv", [KVH, HD], f32, kind="ExternalInput")
    woh = nc.dram_tensor("woh", [128, (HID // 512) * QH * 512], ddt,
                         kind="ExternalInput")
    ident = nc.dram_tensor("ident", [128, 128], ddt, kind="ExternalInput")
    ones = nc.dram_tensor("ones", [128, 128], ddt, kind="ExternalInput")
    out = nc.dram_tensor("out", [S, HID], mdt, kind="ExternalOutput")

    def bc(ap):
        """View a DRAM fp32 AP as f32r (no-op for bf16)."""
        return ap.bitcast(mdt) if dt == "f32r" else ap

    with tile.TileContext(nc) as tc, ExitStack() as ctx:
        Exp = mybir.ActivationFunctionType.Exp
        Ident = mybir.ActivationFunctionType.Identity

        const = ctx.enter_context(tc.tile_pool(name="const", bufs=1))
        bq_t = const.tile([128, QH], f32)
        bk_t = const.tile([128, KVH], f32)
        bv_t = const.tile([128, KVH], f32)
        id_t = const.tile([128, 128], mdt)
        ones_t = const.tile([128, 128], mdt)
        cos_t = const.tile([128, S], mdt)
        sin_t = const.tile([128, S], mdt)

        # Persistent activations (feature-major). attnT overwrites q in place.
        qpool = ctx.enter_context(tc.tile_pool(name="qpool", bufs=1))
        q_t = [qpool.tile([128, S], mdt, name=f"q{h}") for h in range(QH)]
        kvpool = ctx.enter_context(tc.tile_pool(name="kvpool", bufs=1))
        k_t = [kvpool.tile([128, S], mdt, name=f"k{f}") for f in range(KVH)]
        v_t = kvpool.tile([128, KT * FKV], mdt, name="v")  # [tok%128, (kt, kv*128+d)]

        rope_pool = ctx.enter_context(tc.tile_pool(name="ropep", bufs=1))

        def rope_inplace(x_t, sl, tag):
            """x[:, sl] = x[:, sl]*cos + shift64(x[:, sl])*sin', VectorE + DMA only."""
            w = sl.stop - sl.start
            rot = rope_pool.tile([128, 1024], mdt, name=f"rot_{tag}", tag="rot", bufs=4)
            nc.sync.dma_start(rot[0:HALF, :w], x_t[HALF:128, sl])
            nc.sync.dma_start(rot[HALF:128, :w], x_t[0:HALF, sl])
            t1 = rope_pool.tile([128, 1024], mdt, name=f"t1_{tag}", tag="t1", bufs=4)
            nc.vector.tensor_mul(t1[:, :w], rot[:, :w], sin_t[:, sl])
            nc.vector.tensor_mul(x_t[:, sl], x_t[:, sl], cos_t[:, sl])
            nc.vector.tensor_add(x_t[:, sl], x_t[:, sl], t1[:, :w])

        # Pipeline input pools opened early so chunk-0 prefetch can overlap
        # the V-transpose section.
        hsq = ctx.enter_context(tc.tile_pool(name="hsq", bufs=1))
        wqp = ctx.enter_context(tc.tile_pool(name="wqp", bufs=1))
        QB = min(512, S)
        NCH = S // QB
        NG = KT // 2

        def load_hs_part(c, ks):
            tiles = []
            c0 = c * QB
            for k in ks:
                t = hsq.tile([128, QB], mdt, name=f"hsq_{c}_{k}", tag=f"hsq{k}",
                             bufs=2)
                nc.sync.dma_start(
                    t[:], bc(hsT.ap()[k * 128:(k + 1) * 128, c0:c0 + QB]))
                tiles.append(t)
            return tiles

        def load_hs_chunk(c):
            return load_hs_part(c, range(KH))

        def load_wq(c, h):
            t = wqp.tile([128, HID], mdt, name=f"wq_{c}_{h}", tag="wq", bufs=2)
            # 4 parallel DMAs so the 1MB load spreads across queues
            for q in range(4):
                qs = q * (HID // 4)
                nc.sync.dma_start(
                    t[:, qs:qs + HID // 4],
                    bc(wqh.ap()[:, h * HID + qs:h * HID + qs + HID // 4]))
            return t

        # PE warmup: ~16 matmuls on a zeroed scratch tile run during the
        # initial DMA wait, so the HAM un-throttles before real work arrives.
        with (
            tc.tile_pool(name="warm", bufs=1) as warm,
            tc.tile_pool(name="warmp", bufs=1, space="PSUM") as warmp,
        ):
            wz = warm.tile([128, 512], mdt)
            nc.vector.memset(wz[:], 0.0)
            wps = [warmp.tile([128, 512], f32, name=f"wps{i}") for i in range(2)]
            for i in range(14):
                nc.tensor.matmul(wps[i % 2][:], wz[:, 0:128], wz[:],
                                 start=True, stop=True)

        # ---- Phase 1: K/V projections (feature-major) ----
        vTp = ctx.enter_context(tc.tile_pool(name="vTp", bufs=1))
        if True:
            vT_t = [vTp.tile([128, S], mdt, name=f"vT{f}") for f in range(KVH)]
            with (
                tc.tile_pool(name="st1", bufs=6) as st1,
                tc.tile_pool(name="wkvp", bufs=1) as wkvp,
                tc.tile_pool(name="pkv", bufs=1, space="PSUM") as pkv,
            ):
                wkv_t = [None] * KH
                for tb in range(NTB):
                    tb0 = tb * TBP
                    psk = [[pkv.tile([128, sl.stop - sl.start], f32,
                                     name=f"psk_{tb}_{f}_{j}", tag=f"psk{f}_{j}")
                            for j, sl in enumerate(NB5)] for f in range(KVH)]
                    psv = [[pkv.tile([128, sl.stop - sl.start], f32,
                                     name=f"psv_{tb}_{f}_{j}", tag=f"psv{f}_{j}")
                            for j, sl in enumerate(NB5)] for f in range(KVH)]
                    for k in range(KH):
                        hs_s = st1.tile([128, TBP], mdt, name=f"hs_{tb}_{k}", tag="hs")
                        nc.sync.dma_start(
                            hs_s[:], bc(hsT.ap()[k * 128:(k + 1) * 128, tb0:tb0 + TBP]))
                        if tb == 0:
                            wkv_t[k] = wkvp.tile([128, 2 * FKV], mdt,
                                                 name=f"wkv_{k}")
                            nc.sync.dma_start(wkv_t[k][:],
                                              bc(wkv.ap()[k * 128:(k + 1) * 128, :]))
                        wk_s = wkv_t[k][:, 0:FKV]
                        wv_s = wkv_t[k][:, FKV:2 * FKV]
                        if tb == NTB - 1:
                            # drip-feed chunk-0 prefetch so tb1's own loads
                            # stay ahead in the DMA queues
                            if k == 3:
                                wq_cur = load_wq(0, 0)
                                hs_parts = []
                            if k in (6, 12, 18, 24):
                                hs_parts += load_hs_part(
                                    0, range((k - 6) * 4 // 3, (k - 6) * 4 // 3 + 8))
                        for f in range(KVH):
                            for j, sl in enumerate(NB5):
                                nc.tensor.matmul(psk[f][j][:], wk_s[:, f * 128:(f + 1) * 128],
                                                 hs_s[:, sl], start=(k == 0), stop=(k == KH - 1))
                        for f in range(KVH):
                            for j, sl in enumerate(NB5):
                                nc.tensor.matmul(psv[f][j][:], wv_s[:, f * 128:(f + 1) * 128],
                                                 hs_s[:, sl], start=(k == 0), stop=(k == KH - 1))
                    if tb == 0:
                        nc.sync.dma_start(bq_t[:], bq.ap().rearrange("h p -> p h"))
                        nc.sync.dma_start(bk_t[:], bk.ap().rearrange("h p -> p h"))
                        nc.sync.dma_start(bv_t[:], bv.ap().rearrange("h p -> p h"))
                        nc.sync.dma_start(id_t[:], bc(ident.ap()))
                        nc.sync.dma_start(ones_t[:], bc(ones.ap()))
                        nc.sync.dma_start(cos_t[:], bc(cosT.ap()))
                        nc.sync.dma_start(sin_t[:], bc(sinT.ap()))
                    for f in range(KVH):
                        for j, sl in enumerate(NB5):
                            tsl = slice(tb0 + sl.start, tb0 + sl.stop)
                            nc.scalar.activation(k_t[f][:, tsl], psk[f][j][:], Ident,
                                                 bias=bk_t[:, f:f + 1])
                            nc.vector.tensor_scalar_add(vT_t[f][:, tsl], psv[f][j][:],
                                                        bv_t[:, f:f + 1])
                    # RoPE on this tb's K tokens (PSUM-free; overlaps the next pass)
                    for f in range(KVH):
                        rope_inplace(k_t[f], slice(tb0, tb0 + TBP), f"k{f}_{tb}")

            hs_c = hs_parts

        vv = v_t[:].rearrange("p (kt fkv) -> p kt fkv", fkv=FKV)
        GT = 4 if KT % 4 == 0 else 1
        tgroups = [(f, kt0) for f in range(KVH) for kt0 in range(0, KT, GT)]

        # ---- Phase 3+4+5: chunked Q-proj | attention | o_proj pipeline ----
        with (
            tc.tile_pool(name="expp", bufs=1) as expp,
            tc.tile_pool(name="spool", bufs=1) as spool,
            tc.tile_pool(name="invp", bufs=2) as invp,
            tc.tile_pool(name="pss", bufs=1, space="PSUM") as pss,
            tc.tile_pool(name="pso", bufs=1, space="PSUM") as pso,
            tc.tile_pool(name="aux", bufs=1, space="PSUM") as aux,
        ):
            def attn_begin(h, c):
                po = pso.tile([128, QB], f32, name=f"po_{h}_{c}", tag="oo", bufs=1)
                return {"po": po, "ranks": {}, "h": h, "c": c}

            def attn_group(st, g):
                h, c = st["h"], st["c"]
                f = h // (QH // KVH)
                sl = slice(c * QB, (c + 1) * QB)
                kt0 = 2 * g
                ps = pss.tile([128, 2 * QB], f32, name=f"ps_{h}_{c}_{g}", tag="ss",
                              bufs=2)
                for j in range(2):
                    nc.tensor.matmul(ps[:, j * QB:(j + 1) * QB],
                                     k_t[f][:, (kt0 + j) * 128:(kt0 + j + 1) * 128],
                                     q_t[h][:, sl], start=True, stop=True)
                et = expp.tile([128, 2 * QB], mdt, name=f"e_{h}_{c}_{g}", tag="et",
                               bufs=4)
                nc.scalar.activation(et[:], ps[:], Exp, scale=SCALE)
                for j in range(2):
                    kt = kt0 + j
                    nc.tensor.matmul(st["po"][:],
                                     v_t[:, kt * FKV + f * 128: kt * FKV + (f + 1) * 128],
                                     et[:, j * QB:(j + 1) * QB],
                                     start=(kt == 0), stop=(kt == KT - 1))
                node = spool.tile([128, QB], mdt, name=f"pa_{h}_{c}_{g}",
                                  tag=f"pa{g % 4}", bufs=2)
                nc.vector.tensor_add(node[:], et[:, 0:QB], et[:, QB:2 * QB])
                rank = 1
                ranks = st["ranks"]
                while rank in ranks:
                    prev = ranks.pop(rank)
                    nc.vector.tensor_add(prev[:], prev[:], node[:])
                    node, rank = prev, rank + 1
                ranks[rank] = node

            def attn_final(st):
                h, c = st["h"], st["c"]
                sl = slice(c * QB, (c + 1) * QB)
                rem = [st["ranks"][r] for r in sorted(st["ranks"])]
                ssum = rem[0]
                for other in rem[1:]:
                    nc.vector.tensor_add(ssum[:], ssum[:], other[:])
                pb = aux.tile([128, QB], f32, name=f"pb_{h}_{c}", tag="aux",
                              bufs=1)
                nc.tensor.matmul(pb[:], ones_t[:], ssum[:], start=True, stop=True)
                inv = invp.tile([128, QB], f32, name=f"inv_{h}_{c}", tag="inv")
                nc.vector.reciprocal_approx_fast(inv[:], pb[:])
                nc.vector.tensor_mul(q_t[h][:, sl], st["po"][:], inv[:])

            # -- chunks 0..NCH-1: Q-proj(c) interleaved with attention(c-1) --
            with tc.tile_pool(name="pq", bufs=1, space="PSUM") as pq:
                def transpose_group(gi):
                    # one V-transpose group (4 PE transposes + ScalarE copy)
                    f, kt0 = tgroups[gi]
                    pst = aux.tile([128, GT * 128], mdt, name=f"pst_{f}_{kt0}",
                                   tag="aux", bufs=1)
                    for j in range(GT):
                        nc.tensor.transpose(
                            pst[:, j * 128:(j + 1) * 128],
                            vT_t[f][:, (kt0 + j) * 128:(kt0 + j + 1) * 128], id_t[:])
                    nc.vector.tensor_copy(
                        vv[:, kt0:kt0 + GT, f * 128:(f + 1) * 128],
                        pst[:].rearrange("p (j c) -> p j c", c=128))

                tg_iter = iter(range(len(tgroups)))

                def tg_next():
                    gi = next(tg_iter, None)
                    if gi is not None:
                        transpose_group(gi)

                for c in range(NCH):
                    csl = slice(c * QB, (c + 1) * QB)
                    for h in range(QH):
                        # prefetch next head's weights (or next chunk's head 0)
                        if h + 1 < QH:
                            wq_nxt = load_wq(c, h + 1)
                        elif c + 1 < NCH:
                            wq_nxt = load_wq(c + 1, 0)
                        else:
                            wq_nxt = None
                        psq_t = pq.tile([128, QB], f32, name=f"psq_{c}_{h}",
                                        tag="psq", bufs=2)
                        if c == 0:
                            # V-transposes ride inside chunk 0; front-load two
                            # groups as filler while the prefetch DMAs land
                            if h == 0:
                                tg_next()
                                tg_next()
                            for k in range(KH // 2):
                                nc.tensor.matmul(psq_t[:],
                                                 wq_cur[:, k * 128:(k + 1) * 128],
                                                 hs_c[k][:], start=(k == 0),
                                                 stop=(k == KH - 1))
                            tg_next()
                            for k in range(KH // 2, KH):
                                nc.tensor.matmul(psq_t[:],
                                                 wq_cur[:, k * 128:(k + 1) * 128],
                                                 hs_c[k][:], start=(k == 0),
                                                 stop=(k == KH - 1))
                        else:
                            st = attn_begin(h, c - 1)
                            for g in range(NG):
                                attn_group(st, g)
                                for k in range(4 * g, 4 * g + 4):
                                    nc.tensor.matmul(psq_t[:],
                                                     wq_cur[:, k * 128:(k + 1) * 128],
                                                     hs_c[k][:], start=(k == 0),
                                                     stop=(k == KH - 1))
                            attn_final(st)
                        if h == QH // 2 and c + 1 < NCH:
                            nxt_hs = load_hs_chunk(c + 1)
                        nc.vector.tensor_scalar_add(q_t[h][:, csl], psq_t[:],
                                                    bq_t[:, h:h + 1])
                        rope_inplace(q_t[h], csl, f"q{h}_{c}")
                        wq_cur = wq_nxt
                    if c + 1 < NCH:
                        hs_c = nxt_hs

            # -- final chunk's attention interleaved with o_proj --
            with (
                tc.tile_pool(name="st3", bufs=4) as st3,
                tc.tile_pool(name="osb", bufs=2) as osb,
                tc.tile_pool(name="po5", bufs=1, space="PSUM") as po5,
            ):
                NHB = HID // 512
                NT1 = KT - QB // 128  # token tiles in chunks 0..NCH-2: 12

                def load_woh(i, seq):
                    if i >= len(seq):
                        return None
                    phase, hb = seq[i]
                    w = st3.tile([128, QH * 512], mdt,
                                 name=f"wo_{phase}_{hb}", tag="woh", bufs=2)
                    for q in range(4):
                        qs = q * (QH * 512 // 4)
                        nc.sync.dma_start(
                            w[:, qs:qs + QH * 512 // 4],
                            bc(woh.ap()[:, hb * QH * 512 + qs:
                                        hb * QH * 512 + qs + QH * 512 // 4]))
                    return w

                def oproj_units():
                    seq = [(p, hb) for p in (0, 1) for hb in range(NHB)]
                    tiles = {0: load_woh(0, seq), 1: load_woh(1, seq)}
                    for i, (phase, hb) in enumerate(seq):
                        w = tiles.pop(i)
                        tts = range(NT1) if phase == 0 else range(NT1, KT)
                        for ti, tt in enumerate(tts):
                            if ti == 1 and i + 2 not in tiles:
                                tiles[i + 2] = load_woh(i + 2, seq)
                            pot = po5.tile([128, 512], f32, name=f"pot_{hb}_{tt}",
                                           tag="po", bufs=2)
                            for fh in range(QH):
                                nc.tensor.matmul(pot[:],
                                                 q_t[fh][:, tt * 128:(tt + 1) * 128],
                                                 w[:, fh * 512:(fh + 1) * 512],
                                                 start=(fh == 0), stop=(fh == QH - 1))
                            ot = osb.tile([128, 512], mdt, name=f"ot_{hb}_{tt}",
                                          tag="ot", bufs=4)
                            nc.vector.tensor_copy(ot[:], pot[:])
                            nc.sync.dma_start(
                                out.ap()[tt * 128:(tt + 1) * 128,
                                         hb * 512:(hb + 1) * 512], ot[:])
                            yield

                gen = oproj_units()
                g_idx = 0
                for h in range(QH):
                    st = attn_begin(h, NCH - 1)
                    for g in range(NG):
                        attn_group(st, g)
                        next(gen)
                        if g_idx % 2 == 0:
                            next(gen)   # 96 pass-1 units over 64 groups
                        g_idx += 1
                    attn_final(st)
                for _ in gen:
                    pass

    nc.compile()
    return nc


def make_host_constants():
    ident = np.eye(128, dtype=np.float32)
    ones = np.ones((128, 128), dtype=np.float32)
    return ident, ones


def shard_inputs(hidden_states, cos, sin, Wq, bq, Wk, bk, Wv, bv, Wo, S=S_FULL,
                 dt="bf16"):
    ident, ones = make_host_constants()
    if dt == "bf16":
        import ml_dtypes
        big = ml_dtypes.bfloat16
    else:
        big = np.float32
    in_maps = []
    for c in range(8):
        b, t = c // TP, c % TP
        sinT = np.ascontiguousarray(sin[b].T).astype(np.float32)
        sinT[:HD // 2, :] *= -1.0   # rotate_half sign folded into the table
        wq_slice = Wq[:, t * FQ:(t + 1) * FQ]
        # [p, h*HID + kt*128 + cc] layout: one contiguous DMA per head
        wqh = np.ascontiguousarray(
            wq_slice.reshape(KH, 128, QH, HD).transpose(1, 2, 0, 3).reshape(
                128, QH * HID))
        wo_slice = Wo[t * FQ:(t + 1) * FQ, :]
        # [p, hb*QH*512 + fh*512 + c] layout: one contiguous DMA per hid block
        woh = np.ascontiguousarray(
            wo_slice.reshape(QH, 128, HID // 512, 512).transpose(1, 2, 0, 3)
            .reshape(128, QH * HID))
        m = {
            "hsT": np.ascontiguousarray(hidden_states[b].T).astype(big),
            "cosT": np.ascontiguousarray(cos[b].T).astype(big),
            "sinT": sinT.astype(big),
            "wqh": wqh.astype(big),
            "bq": np.ascontiguousarray(bq[t * FQ:(t + 1) * FQ].reshape(QH, HD)),
            "bk": np.ascontiguousarray(bk[t * FKV:(t + 1) * FKV].reshape(KVH, HD)),
            "bv": np.ascontiguousarray(bv[t * FKV:(t + 1) * FKV].reshape(KVH, HD)),
            "wkv": np.ascontiguousarray(np.concatenate(
                [Wk[:, t * FKV:(t + 1) * FKV], Wv[:, t * FKV:(t + 1) * FKV]],
                axis=1)).astype(big),
            "woh": woh.astype(big),
            "ident": ident.astype(big), "ones": ones.astype(big),
        }
        in_maps.append(m)
    return in_maps


_nc_cache = {}


def kernel(hidden_states, cos, sin, Wq, bq, Wk, bk, Wv, bv, Wo):
    global last_exec_time_ns
    from concourse.bass_utils import run_bass_kernel_spmd

    hidden_states = np.asarray(hidden_states, dtype=np.float32)
    cos = np.asarray(cos, dtype=np.float32)
    sin = np.asarray(sin, dtype=np.float32)
    S = hidden_states.shape[1]
    dt = os.environ.get("ATTN_DT", "bf16")
    if (S, dt) not in _nc_cache:
        _nc_cache[(S, dt)] = build_nc(S, dt)
    nc = _nc_cache[(S, dt)]
    in_maps = shard_inputs(hidden_states, cos, sin,
                           np.asarray(Wq, np.float32), np.asarray(bq, np.float32),
                           np.asarray(Wk, np.float32), np.asarray(bk, np.float32),
                           np.asarray(Wv, np.float32), np.asarray(bv, np.float32),
                           np.asarray(Wo, np.float32), S=S, dt=dt)
    trace = bool(int(os.environ.get("ATTN_TRACE", "0")))
    r = run_bass_kernel_spmd(nc, in_maps, list(range(8)), trace=trace)
    last_exec_time_ns = r.exec_time_ns
    outs = [np.asarray(r.results[c]["out"], dtype=np.float32) for c in range(8)]
    full = np.empty((B, S, HID), dtype=np.float32)
    for b in range(B):
        full[b] = outs[b * TP]
        for t in range(1, TP):
            full[b] += outs[b * TP + t]
    return full


# revision 37
# speedup vs baseline: 1.0122x; 1.0025x over previous
"""Trainium2 Bass kernel for GQA attention block (B=2, S=2048, HID=4096, 32Q/8KV heads).

Sharding: hybrid TP4 x DP2 over 8 NeuronCores.
  core c: batch b = c // 4, TP slice t = c % 4.
  Each core handles one batch element, 8 Q heads (2 KV heads); o_proj partials
  summed on host.

On-chip dataflow is feature-major: qT/kT [head_dim, tokens]; scores come out
keys-major (softmax key-sum via ones-matmul), attention output lands as o_proj
lhsT. RoPE = two SBUF partition-shift DMAs + vector combine (sign folded into
host-negated sin table).

Software pipeline (v2): attention's softmax exp saturates ScalarE (~285us of
EXP for the whole kernel), so Q-projection is interleaved with attention at
score-group granularity: token chunk c's Q-proj matmuls fill the PE while
chunk c-1's attention exp runs on ScalarE. The final chunk's attention
interleaves with o_proj tiles. All Q-evac/rope work rides on VectorE so
ScalarE does exp only. PSUM: pss(4) + po(1) + psb(1) + psq|pot(2) = 8 banks.
"""
import os
import sys

for _p in ("/opt/trn_rl_repo", "/root/.axon_site"):
    if _p not in sys.path and os.path.isdir(_p):
        sys.path.append(_p)

import numpy as np

B, S_FULL, HID = 2, 2048, 4096
NH, NKV, HD = 32, 8, 128
TP = 4                 # tensor-parallel ways
QH = NH // TP          # 8 q heads per core
KVH = NKV // TP        # 2 kv heads per core
FQ = QH * HD           # 1024
FKV = KVH * HD         # 256
KH = HID // 128        # 32 contraction tiles
SCALE = 1.0 / float(np.sqrt(HD))

last_exec_time_ns = None


def build_nc(S: int = S_FULL, dt: str = "bf16"):
    """Build the per-core Bass program (SPMD: same program, per-core inputs)."""
    import concourse.bass as bass
    import concourse.tile as tile
    from concourse import bacc, mybir
    from contextlib import ExitStack

    f32 = mybir.dt.float32
    mdt = mybir.dt.bfloat16 if dt == "bf16" else mybir.dt.float32r
    ddt = mdt if dt == "bf16" else f32   # DRAM dtype for big inputs
    TBP = min(1024, S)                   # kv projection token block
    NTB = S // TBP
    NB5 = [slice(j * 512, min((j + 1) * 512, TBP)) for j in range((TBP + 511) // 512)]
    KT = S // 128                        # key/token tiles
    HALF = HD // 2
    QB = min(512, S)                     # pipeline token chunk
    NCH = S // QB                        # 4 chunks
    NG = KT // 2                         # score groups per (head, chunk): 8

    nc = bacc.Bacc("TRN2", target_bir_lowering=False, debug=False)

    hsT = nc.dram_tensor("hsT", [HID, S], ddt, kind="ExternalInput")
    cosT = nc.dram_tensor("cosT", [HD, S], ddt, kind="ExternalInput")
    sinT = nc.dram_tensor("sinT", [HD, S], ddt, kind="ExternalInput")  # sign-folded
    wqh = nc.dram_tensor("wqh", [128, QH * HID], ddt, kind="ExternalInput")
    bq = nc.dram_tensor("bq", [QH, HD], f32, kind="ExternalInput")
    wkv = nc.dram_tensor("wkv", [HID, 2 * FKV], ddt, kind="ExternalInput")
    bk = nc.dram_tensor("bk", [KVH, HD], f32, kind="ExternalInput")
    bv = nc.dram_tensor("bv", [KVH, HD], f32, kind="ExternalInput")
    woh = nc.dram_tensor("woh", [128, (HID // 512) * QH * 512], ddt,
                         kind="ExternalInput")
    ident = nc.dram_tensor("ident", [128, 128], ddt, kind="ExternalInput")
    ones = nc.dram_tensor("ones", [128, 128], ddt, kind="ExternalInput")
    out = nc.dram_tensor("out", [S, HID], mdt, kind="ExternalOutput")

    def bc(ap):
        """View a DRAM fp32 AP as f32r (no-op for bf16)."""
        return ap.bitcast(mdt) if dt == "f32r" else ap

    with tile.TileContext(nc) as tc, ExitStack() as ctx:
        Exp = mybir.ActivationFunctionType.Exp
        Ident = mybir.ActivationFunctionType.Identity

        const = ctx.enter_context(tc.tile_pool(name="const", bufs=1))
        bq_t = const.tile([128, QH], f32)
        bk_t = const.tile([128, KVH], f32)
        bv_t = const.tile([128, KVH], f32)
        id_t = const.tile([128, 128], mdt)
        ones_t = const.tile([128, 128], mdt)
        cos_t = const.tile([128, S], mdt)
        sin_t = const.tile([128, S], mdt)

        # Persistent activations (feature-major). attnT overwrites q in place.
        qpool = ctx.enter_context(tc.tile_pool(name="qpool", bufs=1))
        q_t = [qpool.tile([128, S], mdt, name=f"q{h}") for h in range(QH)]
        kvpool = ctx.enter_context(tc.tile_pool(name="kvpool", bufs=1))
        k_t = [kvpool.tile([128, S], mdt, name=f"k{f}") for f in range(KVH)]
        v_t = kvpool.tile([128, KT * FKV], mdt, name="v")  # [tok%128, (kt, kv*128+d)]

        rope_pool = ctx.enter_context(tc.tile_pool(name="ropep", bufs=1))

        def rope_inplace(x_t, sl, tag):
            """x[:, sl] = x[:, sl]*cos + shift64(x[:, sl])*sin', VectorE + DMA only."""
            w = sl.stop - sl.start
            rot = rope_pool.tile([128, 1024], mdt, name=f"rot_{tag}", tag="rot", bufs=4)
            nc.sync.dma_start(rot[0:HALF, :w], x_t[HALF:128, sl])
            nc.sync.dma_start(rot[HALF:128, :w], x_t[0:HALF, sl])
            t1 = rope_pool.tile([128, 1024], mdt, name=f"t1_{tag}", tag="t1", bufs=4)
            nc.vector.tensor_mul(t1[:, :w], rot[:, :w], sin_t[:, sl])
            nc.vector.tensor_mul(x_t[:, sl], x_t[:, sl], cos_t[:, sl])
            nc.vector.tensor_add(x_t[:, sl], x_t[:, sl], t1[:, :w])

        # Pipeline input pools opened early so chunk-0 prefetch can overlap
        # the V-transpose section.
        hsq = ctx.enter_context(tc.tile_pool(name="hsq", bufs=1))
        wqp = ctx.enter_context(tc.tile_pool(name="wqp", bufs=1))
        QB = min(512, S)
        NCH = S // QB
        NG = KT // 2

        def load_hs_part(c, ks):
            tiles = []
            c0 = c * QB
            for k in ks:
                t = hsq.tile([128, QB], mdt, name=f"hsq_{c}_{k}", tag=f"hsq{k}",
                             bufs=2)
                nc.sync.dma_start(
                    t[:], bc(hsT.ap()[k * 128:(k + 1) * 128, c0:c0 + QB]))
                tiles.append(t)
            return tiles

        def load_hs_chunk(c):
            return load_hs_part(c, range(KH))

        def load_wq(c, h):
            t = wqp.tile([128, HID], mdt, name=f"wq_{c}_{h}", tag="wq", bufs=2)
            # 4 parallel DMAs so the 1MB load spreads across queues
            for q in range(4):
                qs = q * (HID // 4)
                nc.sync.dma_start(
                    t[:, qs:qs + HID // 4],
                    bc(wqh.ap()[:, h * HID + qs:h * HID + qs + HID // 4]))
            return t

        # PE warmup: ~16 matmuls on a zeroed scratch tile run during the
        # initial DMA wait, so the HAM un-throttles before real work arrives.
        with (
            tc.tile_pool(name="warm", bufs=1) as warm,
            tc.tile_pool(name="warmp", bufs=1, space="PSUM") as warmp,
        ):
            wz = warm.tile([128, 512], mdt)
            nc.vector.memset(wz[:], 0.0)
            wps = [warmp.tile([128, 512], f32, name=f"wps{i}") for i in range(2)]
            for i in range(14):
                nc.tensor.matmul(wps[i % 2][:], wz[:, 0:128], wz[:],
                                 start=True, stop=True)

        # ---- Phase 1: K/V projections (feature-major) ----
        vTp = ctx.enter_context(tc.tile_pool(name="vTp", bufs=1))
        if True:
            vT_t = [vTp.tile([128, S], mdt, name=f"vT{f}") for f in range(KVH)]
            with (
                tc.tile_pool(name="st1", bufs=6) as st1,
                tc.tile_pool(name="wkvp", bufs=1) as wkvp,
                tc.tile_pool(name="pkv", bufs=1, space="PSUM") as pkv,
            ):
                wkv_t = [None] * KH
                for tb in range(NTB):
                    tb0 = tb * TBP
                    psk = [[pkv.tile([128, sl.stop - sl.start], f32,
                                     name=f"psk_{tb}_{f}_{j}", tag=f"psk{f}_{j}")
                            for j, sl in enumerate(NB5)] for f in range(KVH)]
                    psv = [[pkv.tile([128, sl.stop - sl.start], f32,
                                     name=f"psv_{tb}_{f}_{j}", tag=f"psv{f}_{j}")
                            for j, sl in enumerate(NB5)] for f in range(KVH)]
                    for k in range(KH):
                        hs_s = st1.tile([128, TBP], mdt, name=f"hs_{tb}_{k}", tag="hs")
                        nc.sync.dma_start(
                            hs_s[:], bc(hsT.ap()[k * 128:(k + 1) * 128, tb0:tb0 + TBP]))
                        if tb == 0:
                            wkv_t[k] = wkvp.tile([128, 2 * FKV], mdt,
                                                 name=f"wkv_{k}")
                            nc.sync.dma_start(wkv_t[k][:],
                                              bc(wkv.ap()[k * 128:(k + 1) * 128, :]))
                        wk_s = wkv_t[k][:, 0:FKV]
                        wv_s = wkv_t[k][:, FKV:2 * FKV]
                        if tb == NTB - 1:
                            # drip-feed chunk-0 prefetch so tb1's own loads
                            # stay ahead in the DMA queues
                            if k == 3:
                                wq_cur = load_wq(0, 0)
                                hs_parts = []
                            if k in (6, 12, 18):
                                hs_parts += load_hs_part(
                                    0, range((k - 6) * 4 // 3, (k - 6) * 4 // 3 + 8))
                        for f in range(KVH):
                            for j, sl in enumerate(NB5):
                                nc.tensor.matmul(psk[f][j][:], wk_s[:, f * 128:(f + 1) * 128],
                                                 hs_s[:, sl], start=(k == 0), stop=(k == KH - 1))
                        for f in range(KVH):
                            for j, sl in enumerate(NB5):
                                nc.tensor.matmul(psv[f][j][:], wv_s[:, f * 128:(f + 1) * 128],
                                                 hs_s[:, sl], start=(k == 0), stop=(k == KH - 1))
                    if tb == 0:
                        nc.sync.dma_start(bq_t[:], bq.ap().rearrange("h p -> p h"))
                        nc.sync.dma_start(bk_t[:], bk.ap().rearrange("h p -> p h"))
                        nc.sync.dma_start(bv_t[:], bv.ap().rearrange("h p -> p h"))
                        nc.sync.dma_start(id_t[:], bc(ident.ap()))
                        nc.sync.dma_start(ones_t[:], bc(ones.ap()))
                        nc.sync.dma_start(cos_t[:], bc(cosT.ap()))
                        nc.sync.dma_start(sin_t[:], bc(sinT.ap()))
                    for f in range(KVH):
                        for j, sl in enumerate(NB5):
                            tsl = slice(tb0 + sl.start, tb0 + sl.stop)
                            nc.scalar.activation(k_t[f][:, tsl], psk[f][j][:], Ident,
                                                 bias=bk_t[:, f:f + 1])
                            nc.vector.tensor_scalar_add(vT_t[f][:, tsl], psv[f][j][:],
                                                        bv_t[:, f:f + 1])
                    # RoPE on this tb's K tokens (PSUM-free; overlaps the next pass)
                    for f in range(KVH):
                        rope_inplace(k_t[f], slice(tb0, tb0 + TBP), f"k{f}_{tb}")
                    if tb == NTB - 1:
                        # final prefetch batch after the rope shift DMAs so
                        # they aren't queued behind it
                        hs_parts += load_hs_part(0, range(24, KH))

            hs_c = hs_parts

        vv = v_t[:].rearrange("p (kt fkv) -> p kt fkv", fkv=FKV)
        GT = 4 if KT % 4 == 0 else 1
        tgroups = [(f, kt0) for f in range(KVH) for kt0 in range(0, KT, GT)]

        # ---- Phase 3+4+5: chunked Q-proj | attention | o_proj pipeline ----
        with (
            tc.tile_pool(name="expp", bufs=1) as expp,
            tc.tile_pool(name="spool", bufs=1) as spool,
            tc.tile_pool(name="invp", bufs=2) as invp,
            tc.tile_pool(name="pss", bufs=1, space="PSUM") as pss,
            tc.tile_pool(name="pso", bufs=1, space="PSUM") as pso,
            tc.tile_pool(name="aux", bufs=1, space="PSUM") as aux,
        ):
            def attn_begin(h, c):
                po = pso.tile([128, QB], f32, name=f"po_{h}_{c}", tag="oo", bufs=1)
                return {"po": po, "ranks": {}, "h": h, "c": c}

            def attn_group(st, g):
                h, c = st["h"], st["c"]
                f = h // (QH // KVH)
                sl = slice(c * QB, (c + 1) * QB)
                kt0 = 2 * g
                ps = pss.tile([128, 2 * QB], f32, name=f"ps_{h}_{c}_{g}", tag="ss",
                              bufs=2)
                for j in range(2):
                    nc.tensor.matmul(ps[:, j * QB:(j + 1) * QB],
                                     k_t[f][:, (kt0 + j) * 128:(kt0 + j + 1) * 128],
                                     q_t[h][:, sl], start=True, stop=True)
                et = expp.tile([128, 2 * QB], mdt, name=f"e_{h}_{c}_{g}", tag="et",
                               bufs=4)
                nc.scalar.activation(et[:], ps[:], Exp, scale=SCALE)
                for j in range(2):
                    kt = kt0 + j
                    nc.tensor.matmul(st["po"][:],
                                     v_t[:, kt * FKV + f * 128: kt * FKV + (f + 1) * 128],
                                     et[:, j * QB:(j + 1) * QB],
                                     start=(kt == 0), stop=(kt == KT - 1))
                node = spool.tile([128, QB], mdt, name=f"pa_{h}_{c}_{g}",
                                  tag=f"pa{g % 4}", bufs=2)
                nc.vector.tensor_add(node[:], et[:, 0:QB], et[:, QB:2 * QB])
                rank = 1
                ranks = st["ranks"]
                while rank in ranks:
                    prev = ranks.pop(rank)
                    nc.vector.tensor_add(prev[:], prev[:], node[:])
                    node, rank = prev, rank + 1
                ranks[rank] = node

            def attn_final(st):
                h, c = st["h"], st["c"]
                sl = slice(c * QB, (c + 1) * QB)
                rem = [st["ranks"][r] for r in sorted(st["ranks"])]
                ssum = rem[0]
                for other in rem[1:]:
                    nc.vector.tensor_add(ssum[:], ssum[:], other[:])
                pb = aux.tile([128, QB], f32, name=f"pb_{h}_{c}", tag="aux",
                              bufs=1)
                nc.tensor.matmul(pb[:], ones_t[:], ssum[:], start=True, stop=True)
                inv = invp.tile([128, QB], f32, name=f"inv_{h}_{c}", tag="inv")
                nc.vector.reciprocal_approx_fast(inv[:], pb[:])
                nc.vector.tensor_mul(q_t[h][:, sl], st["po"][:], inv[:])

            # -- chunks 0..NCH-1: Q-proj(c) interleaved with attention(c-1) --
            with tc.tile_pool(name="pq", bufs=1, space="PSUM") as pq:
                def transpose_group(gi):
                    # one V-transpose group (4 PE transposes + ScalarE copy)
                    f, kt0 = tgroups[gi]
                    pst = aux.tile([128, GT * 128], mdt, name=f"pst_{f}_{kt0}",
                                   tag="aux", bufs=1)
                    for j in range(GT):
                        nc.tensor.transpose(
                            pst[:, j * 128:(j + 1) * 128],
                            vT_t[f][:, (kt0 + j) * 128:(kt0 + j + 1) * 128], id_t[:])
                    nc.vector.tensor_copy(
                        vv[:, kt0:kt0 + GT, f * 128:(f + 1) * 128],
                        pst[:].rearrange("p (j c) -> p j c", c=128))

                tg_iter = iter(range(len(tgroups)))

                def tg_next():
                    gi = next(tg_iter, None)
                    if gi is not None:
                        transpose_group(gi)

                for c in range(NCH):
                    csl = slice(c * QB, (c + 1) * QB)
                    for h in range(QH):
                        # prefetch next head's weights (or next chunk's head 0)
                        if h + 1 < QH:
                            wq_nxt = load_wq(c, h + 1)
                        elif c + 1 < NCH:
                            wq_nxt = load_wq(c + 1, 0)
                        else:
                            wq_nxt = None
                        psq_t = pq.tile([128, QB], f32, name=f"psq_{c}_{h}",
                                        tag="psq", bufs=2)
                        if c == 0:
                            for k in range(KH):
                                nc.tensor.matmul(psq_t[:],
                                                 wq_cur[:, k * 128:(k + 1) * 128],
                                                 hs_c[k][:], start=(k == 0),
                                                 stop=(k == KH - 1))
                            # one V-transpose group per head, after the qproj
                            # matmuls so its waits hide under later heads
                            tg_next()
                        else:
                            st = attn_begin(h, c - 1)
                            for g in range(NG):
                                attn_group(st, g)
                                for k in range(4 * g, 4 * g + 4):
                                    nc.tensor.matmul(psq_t[:],
                                                     wq_cur[:, k * 128:(k + 1) * 128],
                                                     hs_c[k][:], start=(k == 0),
                                                     stop=(k == KH - 1))
                            attn_final(st)
                        if h == QH // 2 and c + 1 < NCH:
                            nxt_hs = load_hs_chunk(c + 1)
                        nc.vector.tensor_scalar_add(q_t[h][:, csl], psq_t[:],
                                                    bq_t[:, h:h + 1])
                        rope_inplace(q_t[h], csl, f"q{h}_{c}")
                        wq_cur = wq_nxt
                    if c + 1 < NCH:
                        hs_c = nxt_hs

            # -- final chunk's attention interleaved with o_proj --
            with (
                tc.tile_pool(name="st3", bufs=4) as st3,
                tc.tile_pool(name="osb", bufs=2) as osb,
                tc.tile_pool(name="po5", bufs=1, space="PSUM") as po5,
            ):
                NHB = HID // 512
                NT1 = KT - QB // 128  # token tiles in chunks 0..NCH-2: 12

                def load_woh(i, seq):
                    if i >= len(seq):
                        return None
                    phase, hb = seq[i]
                    w = st3.tile([128, QH * 512], mdt,
                                 name=f"wo_{phase}_{hb}", tag="woh", bufs=2)
                    for q in range(4):
                        qs = q * (QH * 512 // 4)
                        nc.sync.dma_start(
                            w[:, qs:qs + QH * 512 // 4],
                            bc(woh.ap()[:, hb * QH * 512 + qs:
                                        hb * QH * 512 + qs + QH * 512 // 4]))
                    return w

                def oproj_units():
                    seq = [(p, hb) for p in (0, 1) for hb in range(NHB)]
                    tiles = {0: load_woh(0, seq), 1: load_woh(1, seq)}
                    for i, (phase, hb) in enumerate(seq):
                        w = tiles.pop(i)
                        tts = range(NT1) if phase == 0 else range(NT1, KT)
                        for ti, tt in enumerate(tts):
                            if ti == 1 and i + 2 not in tiles:
                                tiles[i + 2] = load_woh(i + 2, seq)
                            pot = po5.tile([128, 512], f32, name=f"pot_{hb}_{tt}",
                                           tag="po", bufs=2)
                            for fh in range(QH):
                                nc.tensor.matmul(pot[:],
                                                 q_t[fh][:, tt * 128:(tt + 1) * 128],
                                                 w[:, fh * 512:(fh + 1) * 512],
                                                 start=(fh == 0), stop=(fh == QH - 1))
                            ot = osb.tile([128, 512], mdt, name=f"ot_{hb}_{tt}",
                                          tag="ot", bufs=4)
                            nc.vector.tensor_copy(ot[:], pot[:])
                            nc.sync.dma_start(
                                out.ap()[tt * 128:(tt + 1) * 128,
                                         hb * 512:(hb + 1) * 512], ot[:])
                            yield

                gen = oproj_units()
                g_idx = 0
                for h in range(QH):
                    st = attn_begin(h, NCH - 1)
                    for g in range(NG):
                        attn_group(st, g)
                        next(gen)
                        if g_idx % 2 == 0:
                            next(gen)   # 96 pass-1 units over 64 groups
                        g_idx += 1
                    attn_final(st)
                for _ in gen:
                    pass

    nc.compile()
    return nc


def make_host_constants():
    ident = np.eye(128, dtype=np.float32)
    ones = np.ones((128, 128), dtype=np.float32)
    return ident, ones


def shard_inputs(hidden_states, cos, sin, Wq, bq, Wk, bk, Wv, bv, Wo, S=S_FULL,
                 dt="bf16"):
    ident, ones = make_host_constants()
    if dt == "bf16":
        import ml_dtypes
        big = ml_dtypes.bfloat16
    else:
        big = np.float32
    in_maps = []
    for c in range(8):
        b, t = c // TP, c % TP
        sinT = np.ascontiguousarray(sin[b].T).astype(np.float32)
        sinT[:HD // 2, :] *= -1.0   # rotate_half sign folded into the table
        wq_slice = Wq[:, t * FQ:(t + 1) * FQ]
        # [p, h*HID + kt*128 + cc] layout: one contiguous DMA per head
        wqh = np.ascontiguousarray(
            wq_slice.reshape(KH, 128, QH, HD).transpose(1, 2, 0, 3).reshape(
                128, QH * HID))
        wo_slice = Wo[t * FQ:(t + 1) * FQ, :]
        # [p, hb*QH*512 + fh*512 + c] layout: one contiguous DMA per hid block
        woh = np.ascontiguousarray(
            wo_slice.reshape(QH, 128, HID // 512, 512).transpose(1, 2, 0, 3)
            .reshape(128, QH * HID))
        m = {
            "hsT": np.ascontiguousarray(hidden_states[b].T).astype(big),
            "cosT": np.ascontiguousarray(cos[b].T).astype(big),
            "sinT": sinT.astype(big),
            "wqh": wqh.astype(big),
            "bq": np.ascontiguousarray(bq[t * FQ:(t + 1) * FQ].reshape(QH, HD)),
            "bk": np.ascontiguousarray(bk[t * FKV:(t + 1) * FKV].reshape(KVH, HD)),
            "bv": np.ascontiguousarray(bv[t * FKV:(t + 1) * FKV].reshape(KVH, HD)),
            "wkv": np.ascontiguousarray(np.concatenate(
                [Wk[:, t * FKV:(t + 1) * FKV], Wv[:, t * FKV:(t + 1) * FKV]],
                axis=1)).astype(big),
            "woh": woh.astype(big),
            "ident": ident.astype(big), "ones": ones.astype(big),
        }
        in_maps.append(m)
    return in_maps


_nc_cache = {}


def kernel(hidden_states, cos, sin, Wq, bq, Wk, bk, Wv, bv, Wo):
    global last_exec_time_ns
    from concourse.bass_utils import run_bass_kernel_spmd

    hidden_states = np.asarray(hidden_states, dtype=np.float32)
    cos = np.asarray(cos, dtype=np.float32)
    sin = np.asarray(sin, dtype=np.float32)
    S = hidden_states.shape[1]
    dt = os.environ.get("ATTN_DT", "bf16")
    if (S, dt) not in _nc_cache:
        _nc_cache[(S, dt)] = build_nc(S, dt)
    nc = _nc_cache[(S, dt)]
    in_maps = shard_inputs(hidden_states, cos, sin,
                           np.asarray(Wq, np.float32), np.asarray(bq, np.float32),
                           np.asarray(Wk, np.float32), np.asarray(bk, np.float32),
                           np.asarray(Wv, np.float32), np.asarray(bv, np.float32),
                           np.asarray(Wo, np.float32), S=S, dt=dt)
    trace = bool(int(os.environ.get("ATTN_TRACE", "0")))
    r = run_bass_kernel_spmd(nc, in_maps, list(range(8)), trace=trace)
    last_exec_time_ns = r.exec_time_ns
    outs = [np.asarray(r.results[c]["out"], dtype=np.float32) for c in range(8)]
    full = np.empty((B, S, HID), dtype=np.float32)
    for b in range(B):
        full[b] = outs[b * TP]
        for t in range(1, TP):
            full[b] += outs[b * TP + t]
    return full
